# revision 1
# baseline (speedup 1.0000x reference)
"""CSPN (convolutional spatial propagation) kernel for 8 Trainium2 NeuronCores.

Problem: affinity-net 3x3 conv (32->8 ch) -> normalized 9-plane kernel ->
12 iterations of spatially-varying 3x3 propagation on x.

Sharding: 8 cores = (batch b in 0..3) x (H half). Each core owns 240 output
rows plus a 12-row halo on each side (clipped at image edges): 12 iterations
of 3x3 propagation contaminate at most one row per iteration inward from an
artificial slab boundary, so all contaminated rows land in the discarded halo
and no cross-core communication is needed.

Per-core layout:
  - slab = 256 rows (252 real = 240 out + 2x12 halo-but-clipped..., rows
    252..255 dead/zero), stored 2 rows per partition across 128 partitions.
  - x buffer xa[p] = slab rows 2p-1..2p+2 (1 halo row above/below the pair),
    644 cols (2 zero pad each side of the 640 image cols). All 9 propagation
    taps become free-axis offsets; the duplicated halo rows are refreshed by
    two partition-remap SBUF->SBUF DMAs per iteration.
  - conv: block-diagonal stationary K=128 (4 row-blocks x 32 in-ch), M=32
    (4 blocks x 8 aff-ch), 9 float32r matmuls accumulated in PSUM with
    row/col-shifted moving APs; ACT applies the bias on the PSUM->SBUF copy.
  - normalization: shuffle-DMA of aff to a pixels-on-partitions layout,
    then DVE abs-reduce / reciprocal / broadcast-multiply.
"""

import os
import sys

sys.path.insert(0, "/opt/trn_rl_repo")

import numpy as np

import concourse.bass as bass
import concourse.bacc as bacc
import concourse.tile as tile
from concourse import mybir
from contextlib import ExitStack

F32 = mybir.dt.float32
F32R = mybir.dt.float32r

B, C, H, W = 4, 32, 480, 640
OUTR = 240          # output rows per core
HALO = 12
REAL = 252          # real slab rows (240 + clipped halos)
SLAB = 256          # padded slab rows (4 dead)
NPART = 128         # SLAB / 2
WP = 644            # padded x width (2 each side)
WK = 642            # padded kx width (1 each side)
R_CHUNK = 16        # conv rows per chunk (per block)
NCHUNK = 4          # 64 / R_CHUNK
ITER = 12

# offsets in reference order: product([0,1,-1], repeat=2)
OFFSETS = [(oi, oj) for oi in (0, 1, -1) for oj in (0, 1, -1)]


def _build_program():
    nc = bacc.Bacc("TRN2", target_bir_lowering=False, debug=False, num_devices=8)

    kxs = nc.declare_dram_parameter("kxs", [NPART, 66, WK], F32, isOutput=False)
    xs = nc.declare_dram_parameter("xs", [NPART, 4, WP], F32, isOutput=False)
    stat = nc.declare_dram_parameter("stat", [9, 128, 32], F32, isOutput=False)
    bias = nc.declare_dram_parameter("bias", [32, 1], F32, isOutput=False)
    out = nc.declare_dram_parameter("out", [REAL, W], F32, isOutput=True)

    with tile.TileContext(nc) as tc:
        with ExitStack() as ctx:
            _emit(ctx, tc, kxs.ap(), xs.ap(), stat.ap(), bias.ap(), out.ap())

    nc.compile()
    return nc


def _emit(ctx, tc, kxs, xs, stat, bias, out):
    nc = tc.nc

    const = ctx.enter_context(tc.tile_pool(name="const", bufs=1))
    stat_sb = const.tile([128, 9, 32], F32R)
    bias_sb = const.tile([32, 1], F32)
    # stat dram [9, 128, 32] -> sbuf [128, 9, 32] (fp32r = same bytes as fp32)
    nc.sync.dma_start(stat_sb[:], stat.rearrange("k p m -> p k m").bitcast(F32R))
    nc.sync.dma_start(bias_sb[:], bias[:])

    afft_pool = ctx.enter_context(tc.tile_pool(name="afft", bufs=1))
    aff_t = afft_pool.tile([NPART, 8, 2, W], F32)  # [part, ch, row-in-pair, col]
    # touch aff_t before the conv pools allocate so its address range is
    # pinned first (avoids a scheduler slot-reuse race with the kx tiles)
    nc.gpsimd.memset(aff_t[:, 0:1, 0:1, 0:1], 0.0)

    dram_pool = ctx.enter_context(tc.tile_pool(name="drm", bufs=1, space="DRAM"))
    aff_d = dram_pool.tile([8, SLAB, W], F32)  # [ch, slab row, col] bounce buffer

    # ---------------- conv: affinity net ----------------
    with tc.tile_pool(name="kxp", bufs=2) as kx_pool, \
         tc.tile_pool(name="affsb", bufs=1) as aff_pool, \
         tc.tile_pool(name="psp", bufs=2, space="PSUM") as ps_pool:
        for ch in range(NCHUNK):
            kxt = kx_pool.tile([128, R_CHUNK + 2, WK], F32R)
            # host pre-blocks kxs as [128 = (4 blk x 32 ci), 66, 642]: one
            # full-width DMA per chunk (partial-partition DMAs lose port BW)
            nc.sync.dma_start(
                kxt[:],
                kxs[:, R_CHUNK * ch:R_CHUNK * ch + R_CHUNK + 2, :].bitcast(F32R),
            )
            aff_sb = aff_pool.tile([32, R_CHUNK, 2, 320], F32)
            for g in range(R_CHUNK // 2):   # 4-bank psum groups: 2r x 2h
                ps = ps_pool.tile([32, 4, 512], F32)
                for sl in range(4):
                    r, h = 2 * g + sl // 2, sl % 2
                    for k in range(9):
                        di, dj = k // 3, k % 3
                        mov = kxt[:, r + di, 320 * h + dj:320 * h + dj + 320]
                        nc.tensor.matmul(
                            ps[:, sl, 0:320],
                            stat_sb[:, k, :],
                            mov,
                            start=(k == 0),
                            stop=(k == 8),
                        )
                # one ACT for all 4 slots: psum -> sbuf with bias add
                nc.scalar.activation(
                    aff_sb[:, 2 * g:2 * g + 2, :, :], ps[:, :, 0:320],
                    mybir.ActivationFunctionType.Identity,
                    bias=bias_sb[:], scale=1.0,
                )
            # stage to DRAM bounce in one DMA: psum M-order is m = 4c + b, so
            # (c, b) merges into one 32-count stride-40960 dst dim
            dst_stage = aff_d[:].rearrange(
                "c (b r) w -> (c b) r w", b=4
            )[:, R_CHUNK * ch:R_CHUNK * ch + R_CHUNK, :]
            nc.scalar.dma_start(
                dst_stage,
                aff_sb[:].rearrange("m r h w -> m r (h w)"),
            )
        # single gather back: aff_t[p, c, rr, w] <- aff_d[c, 2p+rr, w]
        nc.scalar.dma_start(
            aff_t[:],
            aff_d[:].rearrange("c (p rr) w -> p c rr w", rr=2),
        )

    # ---------------- normalization -> kernel planes ----------------
    kplane_pool = ctx.enter_context(tc.tile_pool(name="kpl", bufs=1))
    k_sb = kplane_pool.tile([NPART, 9, 2, WP], F32)

    with tc.tile_pool(name="nrm", bufs=1) as nrm:
        asum = nrm.tile([NPART, 2 * W], F32, tag="asum")
        rcp = nrm.tile([NPART, 2 * W], F32, tag="rcp")
        ssum = nrm.tile([NPART, 2 * W], F32, tag="ssum")
        s_t = nrm.tile([NPART, 2 * W], F32, tag="s_t")

        av = aff_t[:].rearrange("p c rr w -> p (rr w) c")  # ch innermost
        nc.vector.tensor_reduce(
            asum[:], av, axis=mybir.AxisListType.X, op=mybir.AluOpType.add,
            apply_absolute_value=True,
        )
        nc.vector.reciprocal(rcp[:], asum[:])
        nc.vector.tensor_reduce(
            ssum[:], av, axis=mybir.AxisListType.X, op=mybir.AluOpType.add,
        )
        # planes 1..8 = aff * (1/asum)
        rcp_b = (
            rcp[:].rearrange("p (rr w) -> p rr w", rr=2)
            .unsqueeze(1).broadcast_to([NPART, 8, 2, W])
        )
        nc.vector.tensor_tensor(
            k_sb[:, 1:9, :, 0:W], aff_t[:], rcp_b, mybir.AluOpType.mult
        )
        # plane 0 = 1 - sum(aff)/asum
        nc.vector.tensor_tensor(
            s_t[:], ssum[:], rcp[:], mybir.AluOpType.mult
        )
        nc.vector.tensor_scalar(
            k_sb[:, 0, :, 0:W],
            s_t[:].rearrange("p (rr w) -> p rr w", rr=2),
            -1.0, 1.0, mybir.AluOpType.mult, mybir.AluOpType.add,
        )

    # ---------------- propagation ----------------
    xpool = ctx.enter_context(tc.tile_pool(name="xbuf", bufs=1))
    xa = [
        xpool.tile([NPART, 4, WP], F32, tag="xaA", name="xaA"),
        xpool.tile([NPART, 4, WP], F32, tag="xaB", name="xaB"),
    ]
    nc.sync.dma_start(xa[0][:], xs[:])
    nc.sync.dma_start(xa[1][:], xs[:])

    with tc.tile_pool(name="accp", bufs=2) as accp:
        for it in range(ITER):
            cur = xa[it % 2]
            nxt = xa[(it + 1) % 2]
            acc = accp.tile([NPART, 2, WP], F32, tag="acc")
            tmp = accp.tile([NPART, 2, WP], F32, tag="tmp")
            accg = accp.tile([NPART, 2, WP], F32, tag="accg")
            tmpg = accp.tile([NPART, 2, WP], F32, tag="tmpg")
            a_v = acc[0:126, :, 2:2 + W]
            t_v = tmp[0:126, :, 2:2 + W]
            g_v = accg[0:126, :, 2:2 + W]
            tg_v = tmpg[0:126, :, 2:2 + W]

            def xk(k):
                oi, oj = OFFSETS[k]
                return cur[0:126, 1 - oi:3 - oi, 2 - oj:2 - oj + W]

            def kp(k):
                return k_sb[0:126, k, :, 0:W]

            # two parallel accumulation chains: DVE taps 0..5, Pool taps 6..8
            nc.vector.tensor_tensor(a_v, kp(0), xk(0), mybir.AluOpType.mult)
            for k in range(1, 6):
                nc.vector.tensor_tensor(t_v, kp(k), xk(k), mybir.AluOpType.mult)
                nc.vector.tensor_tensor(a_v, a_v, t_v, mybir.AluOpType.add)
            nc.gpsimd.tensor_tensor(g_v, kp(6), xk(6), mybir.AluOpType.mult)
            for k in (7, 8):
                nc.gpsimd.tensor_tensor(tg_v, kp(k), xk(k), mybir.AluOpType.mult)
                nc.gpsimd.tensor_tensor(g_v, g_v, tg_v, mybir.AluOpType.add)
            nc.vector.tensor_tensor(
                nxt[0:126, 1:3, 2:2 + W], a_v, g_v, mybir.AluOpType.add
            )
            # halo refresh (partition-remap DMAs, spread over both HWDGE queues)
            nc.sync.dma_start(nxt[1:128, 0:1, :], nxt[0:127, 2:3, :])
            nc.scalar.dma_start(nxt[0:126, 3:4, :], nxt[1:127, 1:2, :])

    final = xa[ITER % 2]
    nc.sync.dma_start(out.rearrange("(p rr) w -> p rr w", rr=2), final[0:126, 1:3, 2:2 + W])


_CACHE = {}


def _get_program():
    if "nc" not in _CACHE:
        _CACHE["nc"] = _build_program()
    return _CACHE["nc"]


def _host_inputs(kernel_x, x, W_aff, b_aff):
    """Build the 8 per-core input maps."""
    stat = np.zeros((9, 128, 32), np.float32)
    for k in range(9):
        di, dj = k // 3, k % 3
        for b in range(4):
            for c in range(8):
                # psum partition m = 4c + b (c-major merges the staging DMA)
                stat[k, 32 * b:32 * b + 32, 4 * c + b] = W_aff[c, :, di, dj]
    biasv = np.repeat(b_aff.astype(np.float32), 4).reshape(32, 1)

    in_maps = []
    bidx = (64 * np.arange(4))[:, None] + np.arange(66)[None, :]  # [4, 66]
    for core in range(8):
        b, h = core // 2, core % 2
        img0 = 0 if h == 0 else H - REAL  # 0 or 228
        kxp = np.zeros((C, SLAB + 2, WK), np.float32)
        kxp[:, 1:1 + REAL, 1:1 + W] = kernel_x[b, :, img0:img0 + REAL, :]
        # blocked layout [128 = (4 blk x 32 ci), 66, WK]:
        # kxs[32*blk + ci, rr] = slab row (64*blk + rr - 1) of channel ci
        kxs = kxp[:, bidx, :].transpose(1, 0, 2, 3).reshape(NPART, 66, WK)
        xsp = np.zeros((SLAB + 2, WP), np.float32)
        xsp[1:1 + REAL, 2:2 + W] = x[b, 0, img0:img0 + REAL, :]
        idx = (2 * np.arange(NPART))[:, None] + np.arange(4)[None, :]
        xs = xsp[idx]  # [128, 4, WP]
        in_maps.append({
            "kxs": np.ascontiguousarray(kxs),
            "xs": np.ascontiguousarray(xs),
            "stat": stat,
            "bias": biasv,
        })
    return in_maps


def kernel(kernel_x, x, W_aff, b_aff):
    from concourse.bass_utils import run_bass_kernel_spmd

    nc = _get_program()
    in_maps = _host_inputs(
        np.asarray(kernel_x, np.float32), np.asarray(x, np.float32),
        np.asarray(W_aff, np.float32), np.asarray(b_aff, np.float32),
    )
    res = run_bass_kernel_spmd(
        nc, in_maps, core_ids=list(range(8)),
        trace=os.environ.get("CSPN_TRACE", "0") == "1",
    )
    _CACHE["last_results"] = res
    outf = np.zeros((B, 1, H, W), np.float32)
    for core in range(8):
        b, h = core // 2, core % 2
        o = res.results[core]["out"]  # [252, 640]
        if h == 0:
            outf[b, 0, 0:OUTR, :] = o[0:OUTR]
        else:
            outf[b, 0, H - OUTR:H, :] = o[REAL - OUTR:REAL]
    return outf

